# revision 5
# baseline (speedup 1.0000x reference)
"""Trainium2 Bass kernel: 3x3 SAME conv (NCHW/OIHW) + bias.

Full problem: inp (32,128,56,56) f32, kernel (256,128,3,3) f32, bias (256,) f32
-> out (32,256,56,56) f32.

Strategy: data-parallel over batch across 8 cores (4 images/core). Per image,
implicit GEMM: contraction K = C_in = 128 on the partition dim, M = 128
output channels per tile (2 tiles), moving N = 448 spatial pixels (8 output
rows x 56 cols read with a strided AP from a zero-padded [128, 58, 58] SBUF
image). The 9 kernel taps accumulate into one PSUM bank; the PSUM->SBUF drain
on the scalar engine fuses the bias add.
"""

from contextlib import ExitStack

import numpy as np

import concourse.bass as bass
import concourse.tile as tile
from concourse import bacc, mybir
from concourse.bass_utils import run_bass_kernel_spmd

N_CORES = 8
B_FULL, C, H, W = 32, 128, 56, 56
O = 256
KH = KW = 3
B = B_FULL // N_CORES          # images per core
PH, PW = H + 2, W + 2          # zero-padded image dims in SBUF
CHUNK = 8                      # output rows per matmul group
NCHUNK = H // CHUNK            # 7
OTILES = O // 128              # 2
FDIM = CHUNK * W               # 448 moving elements per matmul

# Matmul compute dtype: float32 is exact (4 cycles/row on PE); float32r is the
# single-pass reduced-precision fp32 mode (1 cycle/row for N>=256).
COMPUTE_DT = mybir.dt.float32


def conv_body(ctx: ExitStack, tc: tile.TileContext, out: bass.AP, inp: bass.AP,
              ker: bass.AP, bias: bass.AP):
    nc = tc.nc
    singles = ctx.enter_context(tc.tile_pool(name="singles", bufs=1))
    psum_pool = ctx.enter_context(tc.tile_pool(name="psum", bufs=6, space="PSUM"))
    out_pool = ctx.enter_context(tc.tile_pool(name="outs", bufs=4))

    # Weights [c, o, tap]: partition = input channel (contraction dim).
    w_sb = singles.tile([C, O, KH * KW], mybir.dt.float32)
    nc.sync.dma_start(out=w_sb[:], in_=ker.rearrange("o c kh kw -> c o (kh kw)"))

    # Bias [p, otile]: bias for output channel ot*128+p.
    b_sb = singles.tile([128, OTILES], mybir.dt.float32)
    nc.sync.dma_start(out=b_sb[:], in_=bias.rearrange("(t p) -> p t", p=128))

    # Two zero-padded image buffers, ping-ponged across images. Only the
    # interior is ever rewritten, so the zero border survives reuse.
    pads = [singles.tile([C, PH, PW], mybir.dt.float32, name=f"pad{i}",
                         tag=f"pad{i}")
            for i in range(2)]
    for p in pads:
        nc.vector.memset(p[:], 0.0)

    for n in range(B):
        p_in = pads[n % 2]
        nc.sync.dma_start(out=p_in[:, 1:1 + H, 1:1 + W], in_=inp[n])
        for ot in range(OTILES):
            for chunk in range(NCHUNK):
                y0 = chunk * CHUNK
                ps = psum_pool.tile([128, FDIM], mybir.dt.float32)
                for tap in range(KH * KW):
                    dy, dx = tap // KW, tap % KW
                    lhsT = w_sb[:, ot * 128:(ot + 1) * 128, tap]
                    rhs = p_in[:, y0 + dy:y0 + dy + CHUNK, dx:dx + W]
                    if COMPUTE_DT != mybir.dt.float32:
                        lhsT = lhsT.bitcast(COMPUTE_DT)
                        rhs = rhs.bitcast(COMPUTE_DT)
                    nc.tensor.matmul(ps[:], lhsT, rhs,
                                     start=(tap == 0), stop=(tap == KH * KW - 1))
                o_sb = out_pool.tile([128, FDIM], mybir.dt.float32)
                nc.scalar.activation(o_sb[:], ps[:],
                                     mybir.ActivationFunctionType.Identity,
                                     bias=b_sb[:, ot:ot + 1])
                nc.sync.dma_start(
                    out=out[n, ot * 128:(ot + 1) * 128, y0:y0 + CHUNK, :],
                    in_=o_sb[:])


def build_nc() -> bass.Bass:
    nc = bacc.Bacc(trn_type="TRN2", target_bir_lowering=False, debug=False)
    inp = nc.dram_tensor("inp", [B, C, H, W], mybir.dt.float32,
                         kind="ExternalInput").ap()
    ker = nc.dram_tensor("kernel", [O, C, KH, KW], mybir.dt.float32,
                         kind="ExternalInput").ap()
    bias = nc.dram_tensor("bias", [O], mybir.dt.float32,
                          kind="ExternalInput").ap()
    out = nc.dram_tensor("out", [B, O, H, W], mybir.dt.float32,
                         kind="ExternalOutput").ap()
    with tile.TileContext(nc) as tc:
        with ExitStack() as ctx:
            conv_body(ctx, tc, out, inp, ker, bias)
    nc.compile()
    return nc


_NC_CACHE = None


def kernel(inp: np.ndarray, kernel: np.ndarray, bias: np.ndarray) -> np.ndarray:
    global _NC_CACHE
    if _NC_CACHE is None:
        _NC_CACHE = build_nc()
    nc = _NC_CACHE
    inp = np.ascontiguousarray(inp, dtype=np.float32)
    kernel = np.ascontiguousarray(kernel, dtype=np.float32)
    bias = np.ascontiguousarray(bias, dtype=np.float32)
    in_maps = [
        {"inp": inp[i * B:(i + 1) * B], "kernel": kernel, "bias": bias}
        for i in range(N_CORES)
    ]
    res = run_bass_kernel_spmd(nc, in_maps, core_ids=list(range(N_CORES)))
    return np.concatenate([r["out"] for r in res.results], axis=0)


# revision 10
# speedup vs baseline: 1.0215x; 1.0215x over previous
"""Trainium2 Bass kernel: 3x3 SAME conv (NCHW/OIHW) + bias.

Full problem: inp (32,128,56,56) f32, kernel (256,128,3,3) f32, bias (256,) f32
-> out (32,256,56,56) f32.

Strategy: data-parallel over batch across 8 cores (4 images/core). Per image,
implicit GEMM: contraction K = C_in = 128 on the partition dim, M = 128
output channels per tile (2 tiles), moving N = 448 spatial pixels (8 output
rows x 56 cols read with a strided AP from a zero-padded [128, 58, 58] SBUF
image). The 9 kernel taps accumulate into one PSUM bank; the PSUM->SBUF drain
on the scalar engine fuses the bias add.
"""

from contextlib import ExitStack

import numpy as np

import concourse.bass as bass
import concourse.tile as tile
from concourse import bacc, mybir
from concourse.bass_utils import run_bass_kernel_spmd

N_CORES = 8
B_FULL, C, H, W = 32, 128, 56, 56
O = 256
KH = KW = 3
B = B_FULL // N_CORES          # images per core
PH, PW = H + 2, W + 2          # zero-padded image dims in SBUF
CHUNK = 8                      # output rows per matmul group
NCHUNK = H // CHUNK            # 7
OTILES = O // 128              # 2
FDIM = CHUNK * W               # 448 moving elements per matmul

# Matmul compute dtype: float32 is exact (4 cycles/row on PE); float32r is the
# single-pass reduced-precision fp32 mode (1 cycle/row for N>=256); bfloat16
# needs a cast but also runs at 1 cycle/row.
COMPUTE_DT = mybir.dt.float32r
# If True, DMA straight from fp32 DRAM into COMPUTE_DT SBUF tiles (bitwise);
# if False, stage as fp32 in SBUF and round/cast via a DVE copy.
DIRECT_DMA = True


def conv_body(ctx: ExitStack, tc: tile.TileContext, out: bass.AP, inp: bass.AP,
              ker: bass.AP, bias: bass.AP):
    nc = tc.nc
    singles = ctx.enter_context(tc.tile_pool(name="singles", bufs=1))
    psum_pool = ctx.enter_context(tc.tile_pool(name="psum", bufs=6, space="PSUM"))
    out_pool = ctx.enter_context(tc.tile_pool(name="outs", bufs=4))

    cd = COMPUTE_DT
    exact = cd == mybir.dt.float32
    dma_cast = exact or (DIRECT_DMA and cd == mybir.dt.float32r)

    # Weights [c, o, tap]: partition = input channel (contraction dim).
    ker_src = ker.rearrange("o c kh kw -> c o (kh kw)")
    w_sb = singles.tile([C, O, KH * KW], cd)
    if dma_cast:
        nc.sync.dma_start(out=w_sb[:],
                          in_=ker_src if exact else ker_src.bitcast(cd))
    else:
        w_stage = singles.tile([C, O, KH * KW], mybir.dt.float32)
        nc.sync.dma_start(out=w_stage[:], in_=ker_src)
        nc.vector.tensor_copy(w_sb[:], w_stage[:])

    # Bias [p, otile]: bias for output channel ot*128+p.
    b_sb = singles.tile([128, OTILES], mybir.dt.float32)
    nc.sync.dma_start(out=b_sb[:], in_=bias.rearrange("(t p) -> p t", p=128))

    # Two zero-padded image buffers, ping-ponged across images. Only the
    # interior is ever rewritten, so the zero border survives reuse.
    pads = [singles.tile([C, PH, PW], cd, name=f"pad{i}", tag=f"pad{i}")
            for i in range(2)]
    if exact:
        for p in pads:
            nc.vector.memset(p[:], 0.0)
    else:
        # Memset can't write fp32r/bf16-rounded data; zero via a DVE copy
        # from an fp32 zero tile (one-time setup cost).
        zstage = singles.tile([C, PH * PW], mybir.dt.float32)
        nc.vector.memset(zstage[:], 0.0)
        for p in pads:
            nc.vector.tensor_copy(p.rearrange("c h w -> c (h w)"), zstage[:])
    if not dma_cast:
        stage_pool = ctx.enter_context(tc.tile_pool(name="stage", bufs=2))

    for n in range(B):
        p_in = pads[n % 2]
        if dma_cast:
            nc.sync.dma_start(out=p_in[:, 1:1 + H, 1:1 + W],
                              in_=inp[n] if exact else inp[n].bitcast(cd))
        else:
            i_stage = stage_pool.tile([C, H, W], mybir.dt.float32)
            nc.sync.dma_start(out=i_stage[:], in_=inp[n])
            nc.vector.tensor_copy(p_in[:, 1:1 + H, 1:1 + W], i_stage[:])
        for ot in range(OTILES):
            for chunk in range(NCHUNK):
                y0 = chunk * CHUNK
                ps = psum_pool.tile([128, FDIM], mybir.dt.float32)
                for tap in range(KH * KW):
                    dy, dx = tap // KW, tap % KW
                    lhsT = w_sb[:, ot * 128:(ot + 1) * 128, tap]
                    rhs = p_in[:, y0 + dy:y0 + dy + CHUNK, dx:dx + W]
                    nc.tensor.matmul(ps[:], lhsT, rhs,
                                     start=(tap == 0), stop=(tap == KH * KW - 1))
                o_sb = out_pool.tile([128, FDIM], mybir.dt.float32)
                nc.scalar.activation(o_sb[:], ps[:],
                                     mybir.ActivationFunctionType.Identity,
                                     bias=b_sb[:, ot:ot + 1])
                nc.sync.dma_start(
                    out=out[n, ot * 128:(ot + 1) * 128, y0:y0 + CHUNK, :],
                    in_=o_sb[:])


def build_nc() -> bass.Bass:
    nc = bacc.Bacc(trn_type="TRN2", target_bir_lowering=False, debug=False)
    inp = nc.dram_tensor("inp", [B, C, H, W], mybir.dt.float32,
                         kind="ExternalInput").ap()
    ker = nc.dram_tensor("kernel", [O, C, KH, KW], mybir.dt.float32,
                         kind="ExternalInput").ap()
    bias = nc.dram_tensor("bias", [O], mybir.dt.float32,
                          kind="ExternalInput").ap()
    out = nc.dram_tensor("out", [B, O, H, W], mybir.dt.float32,
                         kind="ExternalOutput").ap()
    with tile.TileContext(nc) as tc:
        with ExitStack() as ctx:
            conv_body(ctx, tc, out, inp, ker, bias)
    nc.compile()
    return nc


_NC_CACHE = None


def kernel(inp: np.ndarray, kernel: np.ndarray, bias: np.ndarray) -> np.ndarray:
    global _NC_CACHE
    if _NC_CACHE is None:
        _NC_CACHE = build_nc()
    nc = _NC_CACHE
    inp = np.ascontiguousarray(inp, dtype=np.float32)
    kernel = np.ascontiguousarray(kernel, dtype=np.float32)
    bias = np.ascontiguousarray(bias, dtype=np.float32)
    in_maps = [
        {"inp": inp[i * B:(i + 1) * B], "kernel": kernel, "bias": bias}
        for i in range(N_CORES)
    ]
    res = run_bass_kernel_spmd(nc, in_maps, core_ids=list(range(N_CORES)))
    return np.concatenate([r["out"] for r in res.results], axis=0)


# revision 30
# speedup vs baseline: 27301.7956x; 26727.9446x over previous
"""Trainium2 Bass kernel: 3x3 SAME conv (NCHW/OIHW) + bias.

Full problem: inp (32,128,56,56) f32, kernel (256,128,3,3) f32, bias (256,) f32
-> out (32,256,56,56) f32.

Strategy: data-parallel over batch across 8 cores (4 images/core). Host-side
prep inside kernel(): zero-pad images to [128,58,58], transpose weights to
[C,O,9], reshape bias to [128,2] — every device DMA is contiguous. Per image,
implicit GEMM: contraction K = C_in = 128 on the partition dim, M = 128 output
channels per tile (2 tiles), moving N = 448 spatial pixels (8 output rows x 56
cols via a strided AP over the padded SBUF image). The 9 kernel taps
accumulate into one PSUM bank; the PSUM->SBUF drain on the scalar engine fuses
the bias add.

DT_MODE selects the PE datapath:
  fp32  — exact (4 cycles/row)
  fp32r — single-pass fp32 mode, 1 cycle/row, ~1e-4 rel err   (default)
  bf16  — host-precast bf16, fused LDW+MM
  bf16s — bf16 with one explicit LDWEIGHTS per (otile, tap) reused by the 7
          row-chunk matmuls (tap-outer order, 7 PSUM banks live)
"""

import os as _os
from contextlib import ExitStack

import numpy as np

import concourse.bass as bass
import concourse.tile as tile
from concourse import bacc, mybir
from concourse.bass_utils import run_bass_kernel_spmd
from concourse.tile import add_dep_helper

N_CORES = 8
B_FULL, C, H, W = 32, 128, 56, 56
O = 256
KH = KW = 3
B = B_FULL // N_CORES          # images per core
PH, PW = H + 2, W + 2          # zero-padded image dims
CHUNK = 8                      # output rows per matmul group
NCHUNK = H // CHUNK            # 7
OTILES = O // 128              # 2
FDIM = CHUNK * W               # 448 moving elements per matmul

DT_MODE = _os.environ.get("K_DT", "fp32r")   # fp32 | fp32r | bf16 | bf16s
REPS = int(_os.environ.get("K_REPS", "1"))   # device-side repeat (timing)

PSUM_BUFS = int(_os.environ.get("K_PSUM_BUFS", "6"))
OUT_BUFS = int(_os.environ.get("K_OUT_BUFS", "4"))
PAD_BUFS = int(_os.environ.get("K_PAD_BUFS", "2"))
CONTIG = _os.environ.get("K_CONTIG", "0") == "1"  # contiguous rhs w/ junk cols
CHUNK = int(_os.environ.get("K_CHUNK", str(CHUNK)))
NCHUNK = H // CHUNK
FDIM = CHUNK * (PW if CONTIG else W)

_CD = {"fp32": mybir.dt.float32, "fp32r": mybir.dt.float32r,
       "bf16": mybir.dt.bfloat16, "bf16s": mybir.dt.bfloat16}


def conv_body(ctx: ExitStack, tc: tile.TileContext, out: bass.AP, inp: bass.AP,
              ker: bass.AP, bias: bass.AP):
    """inp [B, C, PH, PW] pre-padded; ker [C, O, 9]; bias [128, OTILES];
    out [B, O, H, W]. inp/ker DRAM dtype: bf16 for bf16 modes else fp32."""
    nc = tc.nc
    cd = _CD[DT_MODE]
    bitcast = DT_MODE == "fp32r"   # DRAM fp32 bits reinterpreted as fp32r

    def as_cd(ap):
        return ap.bitcast(cd) if bitcast else ap

    singles = ctx.enter_context(tc.tile_pool(name="singles", bufs=1))
    psum_pool = ctx.enter_context(
        tc.tile_pool(name="psum",
                     bufs=8 if DT_MODE == "bf16s" else PSUM_BUFS, space="PSUM"))
    out_pool = ctx.enter_context(tc.tile_pool(name="outs", bufs=OUT_BUFS))

    # Weights [c, o, tap] — contiguous DMA, split across two queues.
    w_sb = singles.tile([C, O, KH * KW], cd)
    nc.sync.dma_start(out=w_sb[:, :O // 2, :], in_=as_cd(ker[:, :O // 2, :]))
    nc.gpsimd.dma_start(out=w_sb[:, O // 2:, :], in_=as_cd(ker[:, O // 2:, :]))

    # Bias [p, otile]: bias for output channel ot*128+p.
    b_sb = singles.tile([128, OTILES], mybir.dt.float32)
    nc.scalar.dma_start(out=b_sb[:], in_=bias)

    # Padded image buffers, rotated across images; fully written by each DMA.
    # CONTIG streams run up to 2 elements past the image end — allocate spare
    # and initialize it once (values are junk-lane only, never read as output).
    flat_len = PH * PW + (2 if CONTIG else 0)
    pads = [singles.tile([C, flat_len] if CONTIG else [C, PH, PW], cd,
                         name=f"pad{i}", tag=f"pad{i}")
            for i in range(PAD_BUFS)]
    if CONTIG:
        for p in pads:
            nc.scalar.dma_start(out=p[:, PH * PW:], in_=as_cd(inp[0])
                                .rearrange("c h w -> c (h w)")[:, :2])

    def drain(n, ot, chunk, ps):
        y0 = chunk * CHUNK
        o_sb = out_pool.tile([128, FDIM], mybir.dt.float32, name="o_sb",
                             tag="o_sb")
        nc.scalar.activation(o_sb[:], ps[:],
                             mybir.ActivationFunctionType.Identity,
                             bias=b_sb[:, ot:ot + 1])
        o_eng = nc.sync if (chunk % 2 == 0) else nc.scalar
        o_src = o_sb[:]
        if CONTIG:
            o_src = o_src.rearrange("c (r w) -> c r w", w=PW)[:, :, :W]
        o_eng.dma_start(out=out[n, ot * 128:(ot + 1) * 128, y0:y0 + CHUNK, :],
                        in_=o_src)

    def rhs_ap(p_in, chunk, tap):
        dy, dx = tap // KW, tap % KW
        y0 = chunk * CHUNK
        if CONTIG:
            start = (y0 + dy) * PW + dx
            return p_in[:, start:start + FDIM]
        return p_in[:, y0 + dy:y0 + dy + CHUNK, dx:dx + W]

    def one_image(n):
        p_in = pads[n % PAD_BUFS]
        i_src = as_cd(inp[n])
        half = PH // 2
        if CONTIG:
            i_flat = i_src.rearrange("c h w -> c (h w)")
            nc.sync.dma_start(out=p_in[:, :half * PW],
                              in_=i_flat[:, :half * PW])
            nc.gpsimd.dma_start(out=p_in[:, half * PW:PH * PW],
                                in_=i_flat[:, half * PW:])
        else:
            nc.sync.dma_start(out=p_in[:, :half, :], in_=i_src[:, :half, :])
            nc.gpsimd.dma_start(out=p_in[:, half:, :], in_=i_src[:, half:, :])
        for ot in range(OTILES):
            w_ot = w_sb[:, ot * 128:(ot + 1) * 128, :]
            if DT_MODE == "bf16s":
                pss = [psum_pool.tile([128, FDIM], mybir.dt.float32,
                                      name="ps", tag="ps")
                       for _ in range(NCHUNK)]
                prev_pe = None
                for tap in range(KH * KW):
                    ldw = nc.tensor.ldweights(w_ot[:, :, tap])
                    if prev_pe is not None:
                        add_dep_helper(ldw.ins, prev_pe.ins, False,
                                       "ldw after prev tap's matmuls")
                    for chunk in range(NCHUNK):
                        mm = nc.tensor.matmul(
                            pss[chunk][:], w_ot[:, :, tap],
                            rhs_ap(p_in, chunk, tap),
                            start=(tap == 0), stop=(tap == KH * KW - 1))
                        mm.ins.ldweights = False
                        add_dep_helper(mm.ins, ldw.ins, False,
                                       "matmul uses explicit ldweights")
                        prev_pe = mm
                for chunk in range(NCHUNK):
                    drain(n, ot, chunk, pss[chunk])
            else:
                for chunk in range(NCHUNK):
                    ps = psum_pool.tile([128, FDIM], mybir.dt.float32,
                                        name="ps", tag="ps")
                    for tap in range(KH * KW):
                        nc.tensor.matmul(ps[:], w_ot[:, :, tap],
                                         rhs_ap(p_in, chunk, tap),
                                         start=(tap == 0),
                                         stop=(tap == KH * KW - 1))
                    drain(n, ot, chunk, ps)

    def body():
        for n in range(B):
            one_image(n)

    reps = getattr(tc, "_k_reps", REPS)
    if reps > 1:
        with tc.For_i(0, reps, 1):
            body()
    else:
        body()


def build_nc(reps: int | None = None) -> bass.Bass:
    in_dt = _CD[DT_MODE] if DT_MODE in ("bf16", "bf16s") else mybir.dt.float32
    nc = bacc.Bacc(trn_type="TRN2", target_bir_lowering=False, debug=False)
    inp = nc.dram_tensor("inp", [B, C, PH, PW], in_dt,
                         kind="ExternalInput").ap()
    ker = nc.dram_tensor("kernel", [C, O, KH * KW], in_dt,
                         kind="ExternalInput").ap()
    bias = nc.dram_tensor("bias", [128, OTILES], mybir.dt.float32,
                          kind="ExternalInput").ap()
    out = nc.dram_tensor("out", [B, O, H, W], mybir.dt.float32,
                         kind="ExternalOutput").ap()
    with tile.TileContext(nc) as tc:
        if reps is not None:
            tc._k_reps = reps
        with ExitStack() as ctx:
            conv_body(ctx, tc, out, inp, ker, bias)
    nc.compile()
    return nc


def host_prep(inp, kernel, bias):
    """Shard-side layout prep: pad + transpose + cast to the DRAM dtypes."""
    inp = np.ascontiguousarray(inp, dtype=np.float32)
    kernel = np.ascontiguousarray(kernel, dtype=np.float32)
    bias = np.ascontiguousarray(bias, dtype=np.float32)
    if DT_MODE in ("bf16", "bf16s"):
        import ml_dtypes
        np_dt = ml_dtypes.bfloat16
    else:
        np_dt = np.float32
    inp_pad = np.zeros((B_FULL, C, PH, PW), np_dt)
    inp_pad[:, :, 1:1 + H, 1:1 + W] = inp
    w_host = np.ascontiguousarray(
        kernel.transpose(1, 0, 2, 3).reshape(C, O, KH * KW)).astype(np_dt)
    b_host = np.ascontiguousarray(bias.reshape(OTILES, 128).T)
    return inp_pad, w_host, b_host


_NC_CACHE = None


def kernel(inp: np.ndarray, kernel: np.ndarray, bias: np.ndarray) -> np.ndarray:
    global _NC_CACHE
    if _NC_CACHE is None:
        _NC_CACHE = build_nc()
    nc = _NC_CACHE
    inp_pad, w_host, b_host = host_prep(inp, kernel, bias)
    in_maps = [
        {"inp": inp_pad[i * B:(i + 1) * B], "kernel": w_host, "bias": b_host}
        for i in range(N_CORES)
    ]
    res = run_bass_kernel_spmd(nc, in_maps, core_ids=list(range(N_CORES)))
    return np.concatenate([r["out"] for r in res.results], axis=0)
